# revision 9
# baseline (speedup 1.0000x reference)
"""Cross-attention kernel for Trainium2, sharded over 8 NeuronCores.

Sharding: core c handles batch b = c // 4 and head-group g = c % 4
(4 of 16 heads, i.e. 256 of 1024 channels). Each core computes
  q_g = query[b] @ Wq[g].T ; k_g = key[b] @ Wk[g].T ; v_g = value[b] @ Wv[g].T
  x_g = softmax(q_g k_g^T * scale) v_g          (4 heads, independent)
  partial_g = x_g @ Wp[:, g].T                  (partial over head-group)
Host sums the 4 partials per batch and adds the bias terms
(bp exactly, and bv folded through: softmax rows sum to 1, so the value
bias contributes bv @ Wp.T to every token).

On-chip layout: channel-major ("transposed") activations so every matmul
contracts along SBUF partitions. Scores are computed transposed
(scoresT[m, q]) so the softmax denominator reduces along partitions via a
ones-augmented value matrix (extra column of 1.0 in v), and the PV matmul
chains directly off the exp output.

All matmul operands are bf16 (PSUM accumulation stays fp32): halves HBM
traffic so the projections are never DMA-starved, and enables fast weight
load. exp runs on the Scalar engine from fp32 PSUM. All inputs are made
SBUF-resident with bulk DMAs issued up front.
"""

import numpy as np

import concourse.bass as bass
import concourse.mybir as mybir
import concourse.tile as tile
from concourse import bacc
from concourse.bass_utils import run_bass_kernel_spmd

B, N, DIM, H, DH = 2, 2048, 1024, 16, 64
NCORES = 8
HG = 4            # head-groups (cores per batch)
HPG = H // HG     # heads per group = 4
CS = DIM // HG    # channels per group = 256
P = 128
KT = DIM // P     # 8 contraction tiles for the projections
NT = N // P       # 16 token tiles
QW = 512          # q-chunk width (PSUM bank limit for fp32 accumulation)
QC = N // QW      # 4 q-chunks

FP32 = mybir.dt.float32
BF16 = mybir.dt.bfloat16
AF = mybir.ActivationFunctionType


def _build(scale: float, add_qk_bias: bool, reps: int = 1,
           loop_reps: int | None = None):
    nc = bacc.Bacc("TRN2", target_bir_lowering=False, debug=False,
                   num_devices=NCORES)

    qT = nc.dram_tensor("qT", [DIM, N], BF16, kind="ExternalInput").ap()
    kT = nc.dram_tensor("kT", [DIM, N], BF16, kind="ExternalInput").ap()
    vT = nc.dram_tensor("vT", [DIM, N], BF16, kind="ExternalInput").ap()
    wq = nc.dram_tensor("wq", [DIM, CS], BF16, kind="ExternalInput").ap()
    wk = nc.dram_tensor("wk", [DIM, CS], BF16, kind="ExternalInput").ap()
    wv = nc.dram_tensor("wv", [DIM, CS], BF16, kind="ExternalInput").ap()
    wp = nc.dram_tensor("wp", [CS, DIM], BF16, kind="ExternalInput").ap()
    bqk = nc.dram_tensor("bqk", [P, 2 * (CS // P)], FP32,
                         kind="ExternalInput").ap()
    out = nc.dram_tensor("out", [DIM, N], FP32, kind="ExternalOutput").ap()

    from contextlib import ExitStack
    with nc.allow_low_precision(reason="bf16 matmul rounding is intended"), \
         tile.TileContext(nc) as tc, ExitStack() as stack:
        wpool = stack.enter_context(tc.tile_pool(name="wpool", bufs=1))
        inres = stack.enter_context(tc.tile_pool(name="inres", bufs=1))
        persist = stack.enter_context(tc.tile_pool(name="persist", bufs=1))
        const = stack.enter_context(tc.tile_pool(name="const", bufs=1))

        # Weights resident in SBUF.
        wq_sb = wpool.tile([P, KT * CS], BF16, tag="wq")
        wk_sb = wpool.tile([P, KT * CS], BF16, tag="wk")
        wv_sb = wpool.tile([P, KT * CS], BF16, tag="wv")
        wp_sb = wpool.tile([P, (CS // P) * DIM], BF16, tag="wp")
        for k in range(KT):
            nc.sync.dma_start(out=wq_sb[:, k * CS:(k + 1) * CS],
                              in_=wq[k * P:(k + 1) * P, :])
            nc.sync.dma_start(out=wk_sb[:, k * CS:(k + 1) * CS],
                              in_=wk[k * P:(k + 1) * P, :])
            nc.sync.dma_start(out=wv_sb[:, k * CS:(k + 1) * CS],
                              in_=wv[k * P:(k + 1) * P, :])
        for k2 in range(CS // P):
            nc.sync.dma_start(out=wp_sb[:, k2 * DIM:(k2 + 1) * DIM],
                              in_=wp[k2 * P:(k2 + 1) * P, :])
        bqk_sb = const.tile([P, 2 * (CS // P)], FP32, tag="bqk")
        if add_qk_bias:
            nc.sync.dma_start(out=bqk_sb[:], in_=bqk[:])
        ones = const.tile([1, DH], BF16, tag="ones")
        nc.vector.memset(ones[:], 1.0)

        # Full activations resident in SBUF, channel-major, bf16.
        qres = [inres.tile([P, N], BF16, tag=f"qres{k}", name=f"qres{k}") for k in range(KT)]
        kres = [inres.tile([P, N], BF16, tag=f"kres{k}", name=f"kres{k}") for k in range(KT)]
        vres = [inres.tile([P, N], BF16, tag=f"vres{k}", name=f"vres{k}") for k in range(KT)]
        # Input DMAs issued from otherwise-idle engine queues so the
        # descriptor issues (~0.6us each) run in parallel with the weight
        # loads on the sync queue instead of serializing behind them.
        for k in range(KT):
            nc.scalar.dma_start(out=qres[k][:], in_=qT[k * P:(k + 1) * P, :])
        for k in range(KT):
            nc.gpsimd.dma_start(out=kres[k][:], in_=kT[k * P:(k + 1) * P, :])
        for k in range(KT):
            nc.gpsimd.dma_start(out=vres[k][:], in_=vT[k * P:(k + 1) * P, :])

        # Projected activations, channel-major, bf16.
        qsb = [persist.tile([P, N], BF16, tag=f"qsb{t}", name=f"qsb{t}") for t in range(2)]
        ksb = [persist.tile([P, N], BF16, tag=f"ksb{t}", name=f"ksb{t}") for t in range(2)]
        # v token-major with a ones column per head: [tok, 4*(64+1)]
        vsb = [persist.tile([P, HPG * (DH + 1)], BF16, tag=f"vsb{t}", name=f"vsb{t}")
               for t in range(NT)]

        from contextlib import nullcontext
        loop_cm = (tc.For_i(0, loop_reps, 1) if loop_reps
                   else nullcontext())
        with loop_cm:
          for rep in range(reps):
            # ---- Phase Q / K: channel-major projections -------------------
            def qk_proj(res, w_sb, dst, bias_col):
                with tc.tile_pool(name="pp", bufs=1, space="PSUM") as pp:
                    pA = pp.tile([P, N], FP32, tag="pA")
                    pB = pp.tile([P, N], FP32, tag="pB")
                    for k in range(KT):
                        for nn in range(QC):
                            nc.tensor.matmul(
                                pA[:, nn * QW:(nn + 1) * QW],
                                w_sb[:, k * CS:k * CS + P],
                                res[k][:, nn * QW:(nn + 1) * QW],
                                start=(k == 0), stop=(k == KT - 1))
                            nc.tensor.matmul(
                                pB[:, nn * QW:(nn + 1) * QW],
                                w_sb[:, k * CS + P:(k + 1) * CS],
                                res[k][:, nn * QW:(nn + 1) * QW],
                                start=(k == 0), stop=(k == KT - 1))
                    for t, pt in enumerate((pA, pB)):
                        if add_qk_bias:
                            nc.vector.tensor_scalar(
                                dst[t][:], pt[:],
                                bqk_sb[:, bias_col + t:bias_col + t + 1], None,
                                mybir.AluOpType.add)
                        else:
                            nc.vector.tensor_copy(dst[t][:], pt[:])

            qk_proj(qres, wq_sb, qsb, 0)
            qk_proj(kres, wk_sb, ksb, CS // P)

            # ---- Phase V: token-major projection --------------------------
            # One PSUM bank per token-tile accumulator (start=True clears the
            # whole bank, so accumulation groups must not share one). 8 banks
            # per pass, two passes.
            with tc.tile_pool(name="pv", bufs=8, space="PSUM") as pvp:
                for half in range(2):
                    pvt = [pvp.tile([P, CS], FP32, tag="pv",
                                    name=f"pv{half}_{t8}_{rep}") for t8 in range(8)]
                    for k in range(KT):
                        for t8 in range(8):
                            tt = half * 8 + t8
                            nc.tensor.matmul(
                                pvt[t8][:],
                                vres[k][:, tt * P:(tt + 1) * P],
                                wv_sb[:, k * CS:(k + 1) * CS],
                                start=(k == 0), stop=(k == KT - 1))
                    for t8 in range(8):
                        tt = half * 8 + t8
                        dst3 = vsb[tt][:].rearrange("p (h c) -> p h c", h=HPG)
                        nc.vector.tensor_copy(
                            dst3[:, :, 0:DH],
                            pvt[t8][:].rearrange("p (h c) -> p h c", h=HPG))
                        nc.vector.memset(dst3[:, :, DH:DH + 1], 1.0)

            # ---- Phase C: attention + output projection, per q-chunk ------
            # Software-pipelined: scores/exp run LAG m-tiles ahead of the PV
            # matmuls so the Scalar engine (the bottleneck: 128 exps of
            # [128,1024]) never starves at hp/qq boundaries. The softmax
            # division chain is DVE+GpSimd only (no PE instruction), and the
            # output projection of chunk qq is injected into chunk qq+1's
            # m-loop, with its PSUM coming from the sc ring.
            LAG = 2
            with tc.tile_pool(name="probs", bufs=LAG + 2) as probs, \
                 tc.tile_pool(name="xq", bufs=2) as xqp, \
                 tc.tile_pool(name="small", bufs=2) as small, \
                 tc.tile_pool(name="ost", bufs=2) as ostp, \
                 tc.tile_pool(name="psc", bufs=3, space="PSUM") as psc, \
                 tc.tile_pool(name="pxt", bufs=1, space="PSUM") as pxt:

                def oproj_tile(xqp_, qq_, mo):
                    qs_ = slice(qq_ * QW, (qq_ + 1) * QW)
                    po = psc.tile([P, 2 * QW], FP32, tag="sc",
                                  name=f"po_{qq_}_{mo}_{rep}")
                    for k2 in range(CS // P):
                        nc.tensor.matmul(
                            po[:, 0:QW],
                            wp_sb[:, k2 * DIM + mo * P:k2 * DIM + (mo + 1) * P],
                            xqp_[k2][:],
                            start=(k2 == 0), stop=(k2 == CS // P - 1))
                    ost = ostp.tile([P, QW], FP32, tag="ost")
                    nc.vector.tensor_copy(ost[:], po[:, 0:QW])
                    nc.sync.dma_start(out=out[mo * P:(mo + 1) * P, qs_],
                                      in_=ost[:])

                def emit_den(xtA, xtB, xq_, hp_):
                    # softmax division: den row -> reciprocal -> broadcast
                    # matmul across the head dim -> multiply. Emitted lazily
                    # (next hp's m==1) so the bc matmul's wait on the DVE
                    # chain never stalls the next scores in the PE FIFO.
                    for xt, off in ((xtA, 0), (xtB, DH)):
                        den = small.tile([1, QW], FP32, tag="den")
                        nc.vector.tensor_copy(den[:], xt[DH:DH + 1, :])
                        rde = small.tile([1, QW], FP32, tag="rde")
                        nc.vector.reciprocal_approx_fast(out=rde[:], in_=den[:])
                        rdr = small.tile([1, QW], BF16, tag="rdr")
                        nc.vector.tensor_copy(rdr[:], rde[:])
                        bc = psc.tile([P, 2 * QW], FP32, tag="sc")
                        nc.tensor.matmul(bc[0:DH, 0:QW], ones[:], rdr[:],
                                         start=True, stop=True)
                        bcs = small.tile([DH, QW], BF16, tag="bcs")
                        nc.vector.tensor_copy(bcs[:], bc[0:DH, 0:QW])
                        nc.vector.tensor_mul(xq_[hp_][off:off + DH, :],
                                             xt[0:DH, :], bcs[:])

                pend_out = None   # xq pair + qq whose out-proj is pending
                pend_den = None   # xt accumulators awaiting the division
                for qq in range(QC):
                    qs = slice(qq * QW, (qq + 1) * QW)
                    xq = [xqp.tile([P, QW], BF16, tag=f"x{t}", name=f"xq{t}_{qq}_{rep}") for t in range(2)]
                    for hp in range(HPG // 2):
                        # heads A = 2*hp (partitions 0:64 of tile hp),
                        # B = 2*hp+1 (partitions 64:128); their K=64 score
                        # matmuls occupy disjoint PE row-groups and run
                        # concurrently, sharing one [128, 1024] psum tile.
                        xtA = pxt.tile([P, QW], FP32, tag="xtA")
                        xtB = pxt.tile([P, QW], FP32, tag="xtB")
                        prq = []
                        for m in range(NT + LAG):
                            if m < NT:
                                sc = psc.tile([P, 2 * QW], FP32, tag="sc")
                                pr = probs.tile([P, 2 * QW], BF16, tag="pr")
                                prq.append(pr)
                                for j, off in ((0, 0), (1, DH)):
                                    nc.tensor.matmul(
                                        sc[:, j * QW:(j + 1) * QW],
                                        ksb[hp][off:off + DH, m * P:(m + 1) * P],
                                        qsb[hp][off:off + DH, qs],
                                        start=True, stop=True,
                                        tile_position=(off, 0))
                                nc.scalar.activation(pr[:], sc[:], AF.Exp,
                                                     scale=scale)
                            if m == 1 and pend_den is not None:
                                emit_den(*pend_den)
                                pend_den = None
                            if hp == 0 and pend_out is not None \
                                    and 4 <= m < 4 + KT:
                                oproj_tile(*pend_out, m - 4)
                                if m - 4 == KT - 1:
                                    pend_out = None
                            if m >= LAG:
                                mm = m - LAG
                                for j, xt, h in ((0, xtA, 2 * hp),
                                                 (1, xtB, 2 * hp + 1)):
                                    nc.tensor.matmul(
                                        xt[0:DH + 1, :],
                                        vsb[mm][:, h * (DH + 1):(h + 1) * (DH + 1)],
                                        prq[mm][:, j * QW:(j + 1) * QW],
                                        start=(mm == 0), stop=(mm == NT - 1))
                        pend_den = (xtA, xtB, xq, hp)
                    pend_out = (xq, qq)
                # tail: last hp's division, then last chunk's out-proj
                emit_den(*pend_den)
                pend_den = None
                for mo in range(KT):
                    oproj_tile(*pend_out, mo)
                pend_out = None

    nc.compile()
    return nc


_CACHE = {}


def _get_program(scale: float, add_qk_bias: bool, reps: int = 1,
                 loop_reps=None):
    key = (scale, add_qk_bias, reps, loop_reps)
    if key not in _CACHE:
        _CACHE[key] = _build(scale, add_qk_bias, reps, loop_reps)
    return _CACHE[key]


def make_in_maps(query, key, value, Wq, bq, Wk, bk, Wv, bv, Wp, bp, scale):
    import ml_dtypes
    bf16 = ml_dtypes.bfloat16
    query = np.asarray(query, np.float32)
    key = np.asarray(key, np.float32)
    value = np.asarray(value, np.float32)
    Wq, Wk, Wv, Wp = (np.asarray(a, np.float32) for a in (Wq, Wk, Wv, Wp))
    bq, bk = np.asarray(bq, np.float32), np.asarray(bk, np.float32)
    qkvT = [np.ascontiguousarray(a.transpose(0, 2, 1)).astype(bf16)
            for a in (query, key, value)]
    in_maps = []
    for c in range(NCORES):
        b, g = c // HG, c % HG
        cs = slice(g * CS, (g + 1) * CS)
        bqk_arr = np.stack([bq[cs].reshape(CS // P, P),
                            bk[cs].reshape(CS // P, P)]).reshape(-1, P).T
        in_maps.append({
            "qT": qkvT[0][b],
            "kT": qkvT[1][b],
            "vT": qkvT[2][b],
            "wq": np.ascontiguousarray(Wq[cs, :].T).astype(bf16),
            "wk": np.ascontiguousarray(Wk[cs, :].T).astype(bf16),
            "wv": np.ascontiguousarray(Wv[cs, :].T).astype(bf16),
            "wp": np.ascontiguousarray(Wp[:, cs].T).astype(bf16),
            "bqk": np.ascontiguousarray(bqk_arr),
        })
    return in_maps


def combine_outputs(results, bv, bp, Wp):
    bv = np.asarray(bv, np.float32)
    bp = np.asarray(bp, np.float32)
    Wp = np.asarray(Wp, np.float32)
    out = np.empty((B, N, DIM), np.float32)
    corr = bp + bv @ Wp.T
    for b in range(B):
        acc = np.asarray(results[b * HG]["out"], np.float32).copy()
        for g in range(1, HG):
            acc += np.asarray(results[b * HG + g]["out"], np.float32)
        out[b] = acc.T + corr
    return out


def kernel(query, key, value, Wq, bq, Wk, bk, Wv, bv, Wp, bp, scale):
    scale_v = float(np.asarray(scale).reshape(-1)[0])
    add_qk_bias = bool(np.any(np.asarray(bq)) or np.any(np.asarray(bk)))
    nc = _get_program(scale_v, add_qk_bias)
    in_maps = make_in_maps(query, key, value, Wq, bq, Wk, bk, Wv, bv,
                           Wp, bp, scale)
    res = run_bass_kernel_spmd(nc, in_maps, list(range(NCORES))).results
    return combine_outputs(res, bv, bp, Wp)


# revision 11
# speedup vs baseline: 1.1673x; 1.1673x over previous
"""Cross-attention kernel for Trainium2, sharded over 8 NeuronCores.

Sharding: core c handles batch b = c // 4 and head-group g = c % 4
(4 of 16 heads, i.e. 256 of 1024 channels). Each core computes
  q_g = query[b] @ Wq[g].T ; k_g = key[b] @ Wk[g].T ; v_g = value[b] @ Wv[g].T
  x_g = softmax(q_g k_g^T * scale) v_g          (4 heads, independent)
  partial_g = x_g @ Wp[:, g].T                  (partial over head-group)
Host sums the 4 partials per batch and adds the bias terms
(bp exactly, and bv folded through: softmax rows sum to 1, so the value
bias contributes bv @ Wp.T to every token).

On-chip layout: channel-major ("transposed") activations so every matmul
contracts along SBUF partitions. Scores are computed transposed
(scoresT[m, q]) so the softmax denominator reduces along partitions via a
ones-augmented value matrix (extra column of 1.0 in v), and the PV matmul
chains directly off the exp output.

All matmul operands are bf16 (PSUM accumulation stays fp32): halves HBM
traffic so the projections are never DMA-starved, and enables fast weight
load. exp runs on the Scalar engine from fp32 PSUM. All inputs are made
SBUF-resident with bulk DMAs issued up front.
"""

import numpy as np

import concourse.bass as bass
import concourse.mybir as mybir
import concourse.tile as tile
from concourse import bacc
from concourse.bass_utils import run_bass_kernel_spmd

B, N, DIM, H, DH = 2, 2048, 1024, 16, 64
NCORES = 8
HG = 4            # head-groups (cores per batch)
HPG = H // HG     # heads per group = 4
CS = DIM // HG    # channels per group = 256
P = 128
KT = DIM // P     # 8 contraction tiles for the projections
NT = N // P       # 16 token tiles
QW = 512          # q-chunk width (PSUM bank limit for fp32 accumulation)
QC = N // QW      # 4 q-chunks

FP32 = mybir.dt.float32
BF16 = mybir.dt.bfloat16
AF = mybir.ActivationFunctionType


def _build(scale: float, add_qk_bias: bool, reps: int = 1,
           loop_reps: int | None = None):
    nc = bacc.Bacc("TRN2", target_bir_lowering=False, debug=False,
                   num_devices=NCORES)

    qT = nc.dram_tensor("qT", [DIM, N], BF16, kind="ExternalInput").ap()
    kT = nc.dram_tensor("kT", [DIM, N], BF16, kind="ExternalInput").ap()
    vT = nc.dram_tensor("vT", [DIM, N], BF16, kind="ExternalInput").ap()
    wq = nc.dram_tensor("wq", [DIM, CS], BF16, kind="ExternalInput").ap()
    wk = nc.dram_tensor("wk", [DIM, CS], BF16, kind="ExternalInput").ap()
    wv = nc.dram_tensor("wv", [DIM, CS], BF16, kind="ExternalInput").ap()
    wp = nc.dram_tensor("wp", [CS, DIM], BF16, kind="ExternalInput").ap()
    bqk = nc.dram_tensor("bqk", [P, 2 * (CS // P)], FP32,
                         kind="ExternalInput").ap()
    out = nc.dram_tensor("out", [DIM, N], FP32, kind="ExternalOutput").ap()

    from contextlib import ExitStack
    with nc.allow_low_precision(reason="bf16 matmul rounding is intended"), \
         tile.TileContext(nc) as tc, ExitStack() as stack:
        wpool = stack.enter_context(tc.tile_pool(name="wpool", bufs=1))
        inres = stack.enter_context(tc.tile_pool(name="inres", bufs=1))
        persist = stack.enter_context(tc.tile_pool(name="persist", bufs=1))
        const = stack.enter_context(tc.tile_pool(name="const", bufs=1))

        # Weights resident in SBUF.
        wq_sb = wpool.tile([P, KT * CS], BF16, tag="wq")
        wk_sb = wpool.tile([P, KT * CS], BF16, tag="wk")
        wv_sb = wpool.tile([P, KT * CS], BF16, tag="wv")
        wp_sb = wpool.tile([P, (CS // P) * DIM], BF16, tag="wp")
        for k in range(KT):
            nc.sync.dma_start(out=wq_sb[:, k * CS:(k + 1) * CS],
                              in_=wq[k * P:(k + 1) * P, :])
            nc.sync.dma_start(out=wk_sb[:, k * CS:(k + 1) * CS],
                              in_=wk[k * P:(k + 1) * P, :])
            nc.sync.dma_start(out=wv_sb[:, k * CS:(k + 1) * CS],
                              in_=wv[k * P:(k + 1) * P, :])
        for k2 in range(CS // P):
            nc.sync.dma_start(out=wp_sb[:, k2 * DIM:(k2 + 1) * DIM],
                              in_=wp[k2 * P:(k2 + 1) * P, :])
        bqk_sb = const.tile([P, 2 * (CS // P)], FP32, tag="bqk")
        if add_qk_bias:
            nc.sync.dma_start(out=bqk_sb[:], in_=bqk[:])
        ones = const.tile([1, DH], BF16, tag="ones")
        nc.vector.memset(ones[:], 1.0)

        # Full activations resident in SBUF, channel-major, bf16.
        qres = [inres.tile([P, N], BF16, tag=f"qres{k}", name=f"qres{k}") for k in range(KT)]
        kres = [inres.tile([P, N], BF16, tag=f"kres{k}", name=f"kres{k}") for k in range(KT)]
        vres = [inres.tile([P, N], BF16, tag=f"vres{k}", name=f"vres{k}") for k in range(KT)]
        # Input DMAs issued from otherwise-idle engine queues so the
        # descriptor issues (~0.6us each) run in parallel with the weight
        # loads on the sync queue instead of serializing behind them.
        for k in range(KT):
            nc.scalar.dma_start(out=qres[k][:], in_=qT[k * P:(k + 1) * P, :])
        for k in range(KT):
            nc.gpsimd.dma_start(out=kres[k][:], in_=kT[k * P:(k + 1) * P, :])
        for k in range(KT):
            nc.gpsimd.dma_start(out=vres[k][:], in_=vT[k * P:(k + 1) * P, :])

        # Projected activations, channel-major, bf16.
        qsb = [persist.tile([P, N], BF16, tag=f"qsb{t}", name=f"qsb{t}") for t in range(2)]
        ksb = [persist.tile([P, N], BF16, tag=f"ksb{t}", name=f"ksb{t}") for t in range(2)]
        # v token-major with a ones column per head: [tok, 4*(64+1)]
        vsb = [persist.tile([P, HPG * (DH + 1)], BF16, tag=f"vsb{t}", name=f"vsb{t}")
               for t in range(NT)]

        from contextlib import nullcontext
        loop_cm = (tc.For_i(0, loop_reps, 1) if loop_reps
                   else nullcontext())
        with loop_cm:
          for rep in range(reps):
            # ---- Phase Q / K: channel-major projections -------------------
            def qk_proj(res, w_sb, dst, bias_col):
                with tc.tile_pool(name="pp", bufs=1, space="PSUM") as pp:
                    pA = pp.tile([P, N], FP32, tag="pA")
                    pB = pp.tile([P, N], FP32, tag="pB")
                    for k in range(KT):
                        for nn in range(QC):
                            nc.tensor.matmul(
                                pA[:, nn * QW:(nn + 1) * QW],
                                w_sb[:, k * CS:k * CS + P],
                                res[k][:, nn * QW:(nn + 1) * QW],
                                start=(k == 0), stop=(k == KT - 1))
                            nc.tensor.matmul(
                                pB[:, nn * QW:(nn + 1) * QW],
                                w_sb[:, k * CS + P:(k + 1) * CS],
                                res[k][:, nn * QW:(nn + 1) * QW],
                                start=(k == 0), stop=(k == KT - 1))
                    for t, pt in enumerate((pA, pB)):
                        if add_qk_bias:
                            nc.vector.tensor_scalar(
                                dst[t][:], pt[:],
                                bqk_sb[:, bias_col + t:bias_col + t + 1], None,
                                mybir.AluOpType.add)
                        else:
                            nc.vector.tensor_copy(dst[t][:], pt[:])

            qk_proj(qres, wq_sb, qsb, 0)
            qk_proj(kres, wk_sb, ksb, CS // P)

            # ---- Phase V: token-major projection --------------------------
            # One PSUM bank per token-tile accumulator (start=True clears the
            # whole bank, so accumulation groups must not share one). 8 banks
            # per pass, two passes.
            with tc.tile_pool(name="pv", bufs=8, space="PSUM") as pvp:
                for half in range(2):
                    pvt = [pvp.tile([P, CS], FP32, tag="pv",
                                    name=f"pv{half}_{t8}_{rep}") for t8 in range(8)]
                    for k in range(KT):
                        for t8 in range(8):
                            tt = half * 8 + t8
                            nc.tensor.matmul(
                                pvt[t8][:],
                                vres[k][:, tt * P:(tt + 1) * P],
                                wv_sb[:, k * CS:(k + 1) * CS],
                                start=(k == 0), stop=(k == KT - 1))
                    for t8 in range(8):
                        tt = half * 8 + t8
                        dst3 = vsb[tt][:].rearrange("p (h c) -> p h c", h=HPG)
                        nc.vector.tensor_copy(
                            dst3[:, :, 0:DH],
                            pvt[t8][:].rearrange("p (h c) -> p h c", h=HPG))
                        nc.vector.memset(dst3[:, :, DH:DH + 1], 1.0)

            # ---- Phase C: attention + output projection, per q-chunk ------
            # Software-pipelined: scores/exp run LAG m-tiles ahead of the PV
            # matmuls so the Scalar engine (the bottleneck: 128 exps of
            # [128,1024]) never starves at hp/qq boundaries. The softmax
            # division chain is DVE+GpSimd only (no PE instruction), and the
            # output projection of chunk qq is injected into chunk qq+1's
            # m-loop, with its PSUM coming from the sc ring.
            LAG = 3
            with tc.tile_pool(name="probs", bufs=LAG + 2) as probs, \
                 tc.tile_pool(name="xq", bufs=2) as xqp, \
                 tc.tile_pool(name="small", bufs=2) as small, \
                 tc.tile_pool(name="ost", bufs=2) as ostp, \
                 tc.tile_pool(name="psc", bufs=3, space="PSUM") as psc, \
                 tc.tile_pool(name="pxt", bufs=1, space="PSUM") as pxt:

                def oproj_tile(xqp_, qq_, mo):
                    qs_ = slice(qq_ * QW, (qq_ + 1) * QW)
                    po = psc.tile([P, 2 * QW], FP32, tag="sc",
                                  name=f"po_{qq_}_{mo}_{rep}")
                    for k2 in range(CS // P):
                        nc.tensor.matmul(
                            po[:, 0:QW],
                            wp_sb[:, k2 * DIM + mo * P:k2 * DIM + (mo + 1) * P],
                            xqp_[k2][:],
                            start=(k2 == 0), stop=(k2 == CS // P - 1))
                    ost = ostp.tile([P, QW], FP32, tag="ost")
                    nc.vector.tensor_copy(ost[:], po[:, 0:QW])
                    nc.sync.dma_start(out=out[mo * P:(mo + 1) * P, qs_],
                                      in_=ost[:])

                def emit_den(xtA, xtB, xq_, hp_):
                    # softmax division: den row -> reciprocal -> broadcast
                    # matmul across the head dim -> multiply. Emitted lazily
                    # (next hp's m==1) so the bc matmul's wait on the DVE
                    # chain never stalls the next scores in the PE FIFO.
                    for xt, off in ((xtA, 0), (xtB, DH)):
                        den = small.tile([1, QW], FP32, tag="den")
                        nc.vector.tensor_copy(den[:], xt[DH:DH + 1, :])
                        rde = small.tile([1, QW], FP32, tag="rde")
                        nc.vector.reciprocal_approx_fast(out=rde[:], in_=den[:])
                        rdr = small.tile([1, QW], BF16, tag="rdr")
                        nc.vector.tensor_copy(rdr[:], rde[:])
                        bc = psc.tile([P, 2 * QW], FP32, tag="sc")
                        nc.tensor.matmul(bc[0:DH, 0:QW], ones[:], rdr[:],
                                         start=True, stop=True)
                        bcs = small.tile([DH, QW], BF16, tag="bcs")
                        nc.vector.tensor_copy(bcs[:], bc[0:DH, 0:QW])
                        nc.vector.tensor_mul(xq_[hp_][off:off + DH, :],
                                             xt[0:DH, :], bcs[:])

                pend_out = None   # xq pair + qq whose out-proj is pending
                pend_den = None   # xt accumulators awaiting the division
                for qq in range(QC):
                    qs = slice(qq * QW, (qq + 1) * QW)
                    xq = [xqp.tile([P, QW], BF16, tag=f"x{t}", name=f"xq{t}_{qq}_{rep}") for t in range(2)]
                    for hp in range(HPG // 2):
                        # heads A = 2*hp (partitions 0:64 of tile hp),
                        # B = 2*hp+1 (partitions 64:128); their K=64 score
                        # matmuls occupy disjoint PE row-groups and run
                        # concurrently, sharing one [128, 1024] psum tile.
                        xtA = pxt.tile([P, QW], FP32, tag="xtA")
                        xtB = pxt.tile([P, QW], FP32, tag="xtB")
                        prq = []
                        for m in range(NT + LAG):
                            if m < NT:
                                sc = psc.tile([P, 2 * QW], FP32, tag="sc")
                                pr = probs.tile([P, 2 * QW], BF16, tag="pr")
                                prq.append(pr)
                                for j, off in ((0, 0), (1, DH)):
                                    nc.tensor.matmul(
                                        sc[:, j * QW:(j + 1) * QW],
                                        ksb[hp][off:off + DH, m * P:(m + 1) * P],
                                        qsb[hp][off:off + DH, qs],
                                        start=True, stop=True,
                                        tile_position=(off, 0))
                                nc.scalar.activation(pr[:], sc[:], AF.Exp,
                                                     scale=scale)
                            if m == 1 and pend_den is not None:
                                emit_den(*pend_den)
                                pend_den = None
                            if hp == 0 and pend_out is not None \
                                    and 4 <= m < 4 + KT:
                                oproj_tile(*pend_out, m - 4)
                                if m - 4 == KT - 1:
                                    pend_out = None
                            if m >= LAG:
                                mm = m - LAG
                                for j, xt, h in ((0, xtA, 2 * hp),
                                                 (1, xtB, 2 * hp + 1)):
                                    nc.tensor.matmul(
                                        xt[0:DH + 1, :],
                                        vsb[mm][:, h * (DH + 1):(h + 1) * (DH + 1)],
                                        prq[mm][:, j * QW:(j + 1) * QW],
                                        start=(mm == 0), stop=(mm == NT - 1))
                        pend_den = (xtA, xtB, xq, hp)
                    pend_out = (xq, qq)
                # tail: last hp's division, then last chunk's out-proj
                emit_den(*pend_den)
                pend_den = None
                for mo in range(KT):
                    oproj_tile(*pend_out, mo)
                pend_out = None

    nc.compile()
    return nc


_CACHE = {}


def _get_program(scale: float, add_qk_bias: bool, reps: int = 1,
                 loop_reps=None):
    key = (scale, add_qk_bias, reps, loop_reps)
    if key not in _CACHE:
        _CACHE[key] = _build(scale, add_qk_bias, reps, loop_reps)
    return _CACHE[key]


def make_in_maps(query, key, value, Wq, bq, Wk, bk, Wv, bv, Wp, bp, scale):
    import ml_dtypes
    bf16 = ml_dtypes.bfloat16
    query = np.asarray(query, np.float32)
    key = np.asarray(key, np.float32)
    value = np.asarray(value, np.float32)
    Wq, Wk, Wv, Wp = (np.asarray(a, np.float32) for a in (Wq, Wk, Wv, Wp))
    bq, bk = np.asarray(bq, np.float32), np.asarray(bk, np.float32)
    qkvT = [np.ascontiguousarray(a.transpose(0, 2, 1)).astype(bf16)
            for a in (query, key, value)]
    in_maps = []
    for c in range(NCORES):
        b, g = c // HG, c % HG
        cs = slice(g * CS, (g + 1) * CS)
        bqk_arr = np.stack([bq[cs].reshape(CS // P, P),
                            bk[cs].reshape(CS // P, P)]).reshape(-1, P).T
        in_maps.append({
            "qT": qkvT[0][b],
            "kT": qkvT[1][b],
            "vT": qkvT[2][b],
            "wq": np.ascontiguousarray(Wq[cs, :].T).astype(bf16),
            "wk": np.ascontiguousarray(Wk[cs, :].T).astype(bf16),
            "wv": np.ascontiguousarray(Wv[cs, :].T).astype(bf16),
            "wp": np.ascontiguousarray(Wp[:, cs].T).astype(bf16),
            "bqk": np.ascontiguousarray(bqk_arr),
        })
    return in_maps


def combine_outputs(results, bv, bp, Wp):
    bv = np.asarray(bv, np.float32)
    bp = np.asarray(bp, np.float32)
    Wp = np.asarray(Wp, np.float32)
    out = np.empty((B, N, DIM), np.float32)
    corr = bp + bv @ Wp.T
    for b in range(B):
        acc = np.asarray(results[b * HG]["out"], np.float32).copy()
        for g in range(1, HG):
            acc += np.asarray(results[b * HG + g]["out"], np.float32)
        out[b] = acc.T + corr
    return out


def kernel(query, key, value, Wq, bq, Wk, bk, Wv, bv, Wp, bp, scale):
    scale_v = float(np.asarray(scale).reshape(-1)[0])
    add_qk_bias = bool(np.any(np.asarray(bq)) or np.any(np.asarray(bk)))
    nc = _get_program(scale_v, add_qk_bias)
    in_maps = make_in_maps(query, key, value, Wq, bq, Wk, bk, Wv, bv,
                           Wp, bp, scale)
    res = run_bass_kernel_spmd(nc, in_maps, list(range(NCORES))).results
    return combine_outputs(res, bv, bp, Wp)


# revision 12
# speedup vs baseline: 1.1949x; 1.0236x over previous
"""Cross-attention kernel for Trainium2, sharded over 8 NeuronCores.

Sharding: core c handles batch b = c // 4 and head-group g = c % 4
(4 of 16 heads, i.e. 256 of 1024 channels). Each core computes
  q_g = query[b] @ Wq[g].T ; k_g = key[b] @ Wk[g].T ; v_g = value[b] @ Wv[g].T
  x_g = softmax(q_g k_g^T * scale) v_g          (4 heads, independent)
  partial_g = x_g @ Wp[:, g].T                  (partial over head-group)
Host sums the 4 partials per batch and adds the bias terms
(bp exactly, and bv folded through: softmax rows sum to 1, so the value
bias contributes bv @ Wp.T to every token).

On-chip layout: channel-major ("transposed") activations so every matmul
contracts along SBUF partitions. Scores are computed transposed
(scoresT[m, q]) so the softmax denominator reduces along partitions via a
ones-augmented value matrix (extra column of 1.0 in v), and the PV matmul
chains directly off the exp output.

All matmul operands are bf16 (PSUM accumulation stays fp32): halves HBM
traffic so the projections are never DMA-starved, and enables fast weight
load. exp runs on the Scalar engine from fp32 PSUM. All inputs are made
SBUF-resident with bulk DMAs issued up front.
"""

import numpy as np

import concourse.bass as bass
import concourse.mybir as mybir
import concourse.tile as tile
from concourse import bacc
from concourse.bass_utils import run_bass_kernel_spmd

B, N, DIM, H, DH = 2, 2048, 1024, 16, 64
NCORES = 8
HG = 4            # head-groups (cores per batch)
HPG = H // HG     # heads per group = 4
CS = DIM // HG    # channels per group = 256
P = 128
KT = DIM // P     # 8 contraction tiles for the projections
NT = N // P       # 16 token tiles
QW = 512          # q-chunk width (PSUM bank limit for fp32 accumulation)
QC = N // QW      # 4 q-chunks

FP32 = mybir.dt.float32
BF16 = mybir.dt.bfloat16
AF = mybir.ActivationFunctionType


def _build(scale: float, add_qk_bias: bool, reps: int = 1,
           loop_reps: int | None = None):
    nc = bacc.Bacc("TRN2", target_bir_lowering=False, debug=False,
                   num_devices=NCORES)

    qT = nc.dram_tensor("qT", [DIM, N], BF16, kind="ExternalInput").ap()
    kT = nc.dram_tensor("kT", [DIM, N], BF16, kind="ExternalInput").ap()
    vT = nc.dram_tensor("vT", [DIM, N], BF16, kind="ExternalInput").ap()
    wq = nc.dram_tensor("wq", [DIM, CS], BF16, kind="ExternalInput").ap()
    wk = nc.dram_tensor("wk", [DIM, CS], BF16, kind="ExternalInput").ap()
    wv = nc.dram_tensor("wv", [DIM, CS], BF16, kind="ExternalInput").ap()
    wp = nc.dram_tensor("wp", [CS, DIM], BF16, kind="ExternalInput").ap()
    bqk = nc.dram_tensor("bqk", [P, 2 * (CS // P)], FP32,
                         kind="ExternalInput").ap()
    out = nc.dram_tensor("out", [DIM, N], FP32, kind="ExternalOutput").ap()

    from contextlib import ExitStack
    with nc.allow_low_precision(reason="bf16 matmul rounding is intended"), \
         tile.TileContext(nc) as tc, ExitStack() as stack:
        wpool = stack.enter_context(tc.tile_pool(name="wpool", bufs=1))
        inres = stack.enter_context(tc.tile_pool(name="inres", bufs=1))
        persist = stack.enter_context(tc.tile_pool(name="persist", bufs=1))
        const = stack.enter_context(tc.tile_pool(name="const", bufs=1))

        # Weights resident in SBUF.
        wq_sb = wpool.tile([P, KT * CS], BF16, tag="wq")
        wk_sb = wpool.tile([P, KT * CS], BF16, tag="wk")
        wv_sb = wpool.tile([P, KT * CS], BF16, tag="wv")
        wp_sb = wpool.tile([P, (CS // P) * DIM], BF16, tag="wp")
        for k in range(KT):
            nc.sync.dma_start(out=wq_sb[:, k * CS:(k + 1) * CS],
                              in_=wq[k * P:(k + 1) * P, :])
            nc.sync.dma_start(out=wk_sb[:, k * CS:(k + 1) * CS],
                              in_=wk[k * P:(k + 1) * P, :])
            nc.sync.dma_start(out=wv_sb[:, k * CS:(k + 1) * CS],
                              in_=wv[k * P:(k + 1) * P, :])
        for k2 in range(CS // P):
            nc.sync.dma_start(out=wp_sb[:, k2 * DIM:(k2 + 1) * DIM],
                              in_=wp[k2 * P:(k2 + 1) * P, :])
        bqk_sb = const.tile([P, 2 * (CS // P)], FP32, tag="bqk")
        if add_qk_bias:
            nc.sync.dma_start(out=bqk_sb[:], in_=bqk[:])
        ones = const.tile([1, DH], BF16, tag="ones")
        nc.vector.memset(ones[:], 1.0)

        # Full activations resident in SBUF, channel-major, bf16.
        qres = [inres.tile([P, N], BF16, tag=f"qres{k}", name=f"qres{k}") for k in range(KT)]
        kres = [inres.tile([P, N], BF16, tag=f"kres{k}", name=f"kres{k}") for k in range(KT)]
        vres = [inres.tile([P, N], BF16, tag=f"vres{k}", name=f"vres{k}") for k in range(KT)]
        # Input DMAs issued from otherwise-idle engine queues so the
        # descriptor issues (~0.6us each) run in parallel with the weight
        # loads on the sync queue instead of serializing behind them.
        for k in range(KT):
            nc.scalar.dma_start(out=qres[k][:], in_=qT[k * P:(k + 1) * P, :])
        for k in range(KT):
            nc.gpsimd.dma_start(out=kres[k][:], in_=kT[k * P:(k + 1) * P, :])
        for k in range(KT):
            nc.gpsimd.dma_start(out=vres[k][:], in_=vT[k * P:(k + 1) * P, :])

        # Projected activations, channel-major, bf16.
        qsb = [persist.tile([P, N], BF16, tag=f"qsb{t}", name=f"qsb{t}") for t in range(2)]
        ksb = [persist.tile([P, N], BF16, tag=f"ksb{t}", name=f"ksb{t}") for t in range(2)]
        # v token-major with a ones column per head: [tok, 4*(64+1)]
        vsb = [persist.tile([P, HPG * (DH + 1)], BF16, tag=f"vsb{t}", name=f"vsb{t}")
               for t in range(NT)]

        from contextlib import nullcontext
        loop_cm = (tc.For_i(0, loop_reps, 1) if loop_reps
                   else nullcontext())
        with loop_cm:
          for rep in range(reps):
            # ---- Phase Q / K: channel-major projections -------------------
            def qk_proj(res, w_sb, dst, bias_col):
                with tc.tile_pool(name="pp", bufs=1, space="PSUM") as pp:
                    pA = pp.tile([P, N], FP32, tag="pA")
                    pB = pp.tile([P, N], FP32, tag="pB")
                    for k in range(KT):
                        for nn in range(QC):
                            nc.tensor.matmul(
                                pA[:, nn * QW:(nn + 1) * QW],
                                w_sb[:, k * CS:k * CS + P],
                                res[k][:, nn * QW:(nn + 1) * QW],
                                start=(k == 0), stop=(k == KT - 1))
                            nc.tensor.matmul(
                                pB[:, nn * QW:(nn + 1) * QW],
                                w_sb[:, k * CS + P:(k + 1) * CS],
                                res[k][:, nn * QW:(nn + 1) * QW],
                                start=(k == 0), stop=(k == KT - 1))
                    for t, pt in enumerate((pA, pB)):
                        if add_qk_bias:
                            nc.vector.tensor_scalar(
                                dst[t][:], pt[:],
                                bqk_sb[:, bias_col + t:bias_col + t + 1], None,
                                mybir.AluOpType.add)
                        else:
                            nc.vector.tensor_copy(dst[t][:], pt[:])

            qk_proj(kres, wk_sb, ksb, CS // P)

            # ---- Phase V: token-major projection --------------------------
            # One PSUM bank per token-tile accumulator (start=True clears the
            # whole bank, so accumulation groups must not share one). 8 banks
            # per pass, two passes.
            with tc.tile_pool(name="pv", bufs=8, space="PSUM") as pvp:
                for half in range(2):
                    pvt = [pvp.tile([P, CS], FP32, tag="pv",
                                    name=f"pv{half}_{t8}_{rep}") for t8 in range(8)]
                    for k in range(KT):
                        for t8 in range(8):
                            tt = half * 8 + t8
                            nc.tensor.matmul(
                                pvt[t8][:],
                                vres[k][:, tt * P:(tt + 1) * P],
                                wv_sb[:, k * CS:(k + 1) * CS],
                                start=(k == 0), stop=(k == KT - 1))
                    for t8 in range(8):
                        tt = half * 8 + t8
                        dst3 = vsb[tt][:].rearrange("p (h c) -> p h c", h=HPG)
                        nc.vector.tensor_copy(
                            dst3[:, :, 0:DH],
                            pvt[t8][:].rearrange("p (h c) -> p h c", h=HPG))
                        nc.vector.memset(dst3[:, :, DH:DH + 1], 1.0)

            # ---- Phase C: attention + output projection, per q-chunk ------
            # Software-pipelined: scores/exp run LAG m-tiles ahead of the PV
            # matmuls so the Scalar engine (the bottleneck: 128 exps of
            # [128,1024]) never starves at hp/qq boundaries. The softmax
            # division chain is DVE+GpSimd only (no PE instruction), and the
            # output projection of chunk qq is injected into chunk qq+1's
            # m-loop, with its PSUM coming from the sc ring.
            LAG = 3
            with tc.tile_pool(name="probs", bufs=LAG + 2) as probs, \
                 tc.tile_pool(name="xq", bufs=2) as xqp, \
                 tc.tile_pool(name="small", bufs=2) as small, \
                 tc.tile_pool(name="ost", bufs=2) as ostp, \
                 tc.tile_pool(name="psc", bufs=3, space="PSUM") as psc, \
                 tc.tile_pool(name="pxt", bufs=1, space="PSUM") as pxt:

                def oproj_tile(xqp_, qq_, mo):
                    qs_ = slice(qq_ * QW, (qq_ + 1) * QW)
                    po = psc.tile([P, 2 * QW], FP32, tag="sc",
                                  name=f"po_{qq_}_{mo}_{rep}")
                    for k2 in range(CS // P):
                        nc.tensor.matmul(
                            po[:, 0:QW],
                            wp_sb[:, k2 * DIM + mo * P:k2 * DIM + (mo + 1) * P],
                            xqp_[k2][:],
                            start=(k2 == 0), stop=(k2 == CS // P - 1))
                    ost = ostp.tile([P, QW], FP32, tag="ost")
                    nc.vector.tensor_copy(ost[:], po[:, 0:QW])
                    nc.sync.dma_start(out=out[mo * P:(mo + 1) * P, qs_],
                                      in_=ost[:])

                def qproj_chunk(c):
                    # Q projection for one 512-wide chunk, into a psc-ring
                    # tile: lets attention start right after K/V proj, with
                    # later chunks projected inside the previous chunk's
                    # m-loop where the PE has slack.
                    pq = psc.tile([P, 2 * QW], FP32, tag="sc",
                                  name=f"pq_{c}_{rep}")
                    for k in range(KT):
                        nc.tensor.matmul(
                            pq[:, 0:QW],
                            wq_sb[:, k * CS:k * CS + P],
                            qres[k][:, c * QW:(c + 1) * QW],
                            start=(k == 0), stop=(k == KT - 1))
                        nc.tensor.matmul(
                            pq[:, QW:2 * QW],
                            wq_sb[:, k * CS + P:(k + 1) * CS],
                            qres[k][:, c * QW:(c + 1) * QW],
                            start=(k == 0), stop=(k == KT - 1))
                    for t in range(2):
                        dst = qsb[t][:, c * QW:(c + 1) * QW]
                        pqs = pq[:, t * QW:(t + 1) * QW]
                        if add_qk_bias:
                            nc.vector.tensor_scalar(
                                dst, pqs, bqk_sb[:, t:t + 1], None,
                                mybir.AluOpType.add)
                        else:
                            nc.vector.tensor_copy(dst, pqs)

                def emit_den(xtA, xtB, xq_, hp_):
                    # softmax division: den row -> reciprocal -> broadcast
                    # matmul across the head dim -> multiply. Emitted lazily
                    # (next hp's m==1) so the bc matmul's wait on the DVE
                    # chain never stalls the next scores in the PE FIFO.
                    for xt, off in ((xtA, 0), (xtB, DH)):
                        den = small.tile([1, QW], FP32, tag="den")
                        nc.vector.tensor_copy(den[:], xt[DH:DH + 1, :])
                        rde = small.tile([1, QW], FP32, tag="rde")
                        nc.vector.reciprocal_approx_fast(out=rde[:], in_=den[:])
                        rdr = small.tile([1, QW], BF16, tag="rdr")
                        nc.vector.tensor_copy(rdr[:], rde[:])
                        bc = psc.tile([P, 2 * QW], FP32, tag="sc")
                        nc.tensor.matmul(bc[0:DH, 0:QW], ones[:], rdr[:],
                                         start=True, stop=True)
                        bcs = small.tile([DH, QW], BF16, tag="bcs")
                        nc.vector.tensor_copy(bcs[:], bc[0:DH, 0:QW])
                        nc.vector.tensor_mul(xq_[hp_][off:off + DH, :],
                                             xt[0:DH, :], bcs[:])

                pend_out = None   # xq pair + qq whose out-proj is pending
                pend_den = None   # xt accumulators awaiting the division
                qproj_chunk(0)
                for qq in range(QC):
                    qs = slice(qq * QW, (qq + 1) * QW)
                    xq = [xqp.tile([P, QW], BF16, tag=f"x{t}", name=f"xq{t}_{qq}_{rep}") for t in range(2)]
                    for hp in range(HPG // 2):
                        # heads A = 2*hp (partitions 0:64 of tile hp),
                        # B = 2*hp+1 (partitions 64:128); their K=64 score
                        # matmuls occupy disjoint PE row-groups and run
                        # concurrently, sharing one [128, 1024] psum tile.
                        xtA = pxt.tile([P, QW], FP32, tag="xtA")
                        xtB = pxt.tile([P, QW], FP32, tag="xtB")
                        prq = []
                        for m in range(NT + LAG):
                            if m < NT:
                                sc = psc.tile([P, 2 * QW], FP32, tag="sc")
                                pr = probs.tile([P, 2 * QW], BF16, tag="pr")
                                prq.append(pr)
                                for j, off in ((0, 0), (1, DH)):
                                    nc.tensor.matmul(
                                        sc[:, j * QW:(j + 1) * QW],
                                        ksb[hp][off:off + DH, m * P:(m + 1) * P],
                                        qsb[hp][off:off + DH, qs],
                                        start=True, stop=True,
                                        tile_position=(off, 0))
                                nc.scalar.activation(pr[:], sc[:], AF.Exp,
                                                     scale=scale)
                            if m == 1 and pend_den is not None:
                                emit_den(*pend_den)
                                pend_den = None
                            if hp == 0 and pend_out is not None \
                                    and 4 <= m < 4 + KT:
                                oproj_tile(*pend_out, m - 4)
                                if m - 4 == KT - 1:
                                    pend_out = None
                            if hp == 1 and m == 13 and qq + 1 < QC:
                                qproj_chunk(qq + 1)
                            if m >= LAG:
                                mm = m - LAG
                                for j, xt, h in ((0, xtA, 2 * hp),
                                                 (1, xtB, 2 * hp + 1)):
                                    nc.tensor.matmul(
                                        xt[0:DH + 1, :],
                                        vsb[mm][:, h * (DH + 1):(h + 1) * (DH + 1)],
                                        prq[mm][:, j * QW:(j + 1) * QW],
                                        start=(mm == 0), stop=(mm == NT - 1))
                        pend_den = (xtA, xtB, xq, hp)
                    pend_out = (xq, qq)
                # tail: last hp's division, then last chunk's out-proj
                emit_den(*pend_den)
                pend_den = None
                for mo in range(KT):
                    oproj_tile(*pend_out, mo)
                pend_out = None

    nc.compile()
    return nc


_CACHE = {}


def _get_program(scale: float, add_qk_bias: bool, reps: int = 1,
                 loop_reps=None):
    key = (scale, add_qk_bias, reps, loop_reps)
    if key not in _CACHE:
        _CACHE[key] = _build(scale, add_qk_bias, reps, loop_reps)
    return _CACHE[key]


def make_in_maps(query, key, value, Wq, bq, Wk, bk, Wv, bv, Wp, bp, scale):
    import ml_dtypes
    bf16 = ml_dtypes.bfloat16
    query = np.asarray(query, np.float32)
    key = np.asarray(key, np.float32)
    value = np.asarray(value, np.float32)
    Wq, Wk, Wv, Wp = (np.asarray(a, np.float32) for a in (Wq, Wk, Wv, Wp))
    bq, bk = np.asarray(bq, np.float32), np.asarray(bk, np.float32)
    qkvT = [np.ascontiguousarray(a.transpose(0, 2, 1)).astype(bf16)
            for a in (query, key, value)]
    in_maps = []
    for c in range(NCORES):
        b, g = c // HG, c % HG
        cs = slice(g * CS, (g + 1) * CS)
        bqk_arr = np.stack([bq[cs].reshape(CS // P, P),
                            bk[cs].reshape(CS // P, P)]).reshape(-1, P).T
        in_maps.append({
            "qT": qkvT[0][b],
            "kT": qkvT[1][b],
            "vT": qkvT[2][b],
            "wq": np.ascontiguousarray(Wq[cs, :].T).astype(bf16),
            "wk": np.ascontiguousarray(Wk[cs, :].T).astype(bf16),
            "wv": np.ascontiguousarray(Wv[cs, :].T).astype(bf16),
            "wp": np.ascontiguousarray(Wp[:, cs].T).astype(bf16),
            "bqk": np.ascontiguousarray(bqk_arr),
        })
    return in_maps


def combine_outputs(results, bv, bp, Wp):
    bv = np.asarray(bv, np.float32)
    bp = np.asarray(bp, np.float32)
    Wp = np.asarray(Wp, np.float32)
    out = np.empty((B, N, DIM), np.float32)
    corr = bp + bv @ Wp.T
    for b in range(B):
        acc = np.asarray(results[b * HG]["out"], np.float32).copy()
        for g in range(1, HG):
            acc += np.asarray(results[b * HG + g]["out"], np.float32)
        out[b] = acc.T + corr
    return out


def kernel(query, key, value, Wq, bq, Wk, bk, Wv, bv, Wp, bp, scale):
    scale_v = float(np.asarray(scale).reshape(-1)[0])
    add_qk_bias = bool(np.any(np.asarray(bq)) or np.any(np.asarray(bk)))
    nc = _get_program(scale_v, add_qk_bias)
    in_maps = make_in_maps(query, key, value, Wq, bq, Wk, bk, Wv, bv,
                           Wp, bp, scale)
    res = run_bass_kernel_spmd(nc, in_maps, list(range(NCORES))).results
    return combine_outputs(res, bv, bp, Wp)


# revision 13
# speedup vs baseline: 1.2304x; 1.0297x over previous
"""Cross-attention kernel for Trainium2, sharded over 8 NeuronCores.

Sharding: core c handles batch b = c // 4 and head-group g = c % 4
(4 of 16 heads, i.e. 256 of 1024 channels). Each core computes
  q_g = query[b] @ Wq[g].T ; k_g = key[b] @ Wk[g].T ; v_g = value[b] @ Wv[g].T
  x_g = softmax(q_g k_g^T * scale) v_g          (4 heads, independent)
  partial_g = x_g @ Wp[:, g].T                  (partial over head-group)
Host sums the 4 partials per batch and adds the bias terms
(bp exactly, and bv folded through: softmax rows sum to 1, so the value
bias contributes bv @ Wp.T to every token).

On-chip layout: channel-major ("transposed") activations so every matmul
contracts along SBUF partitions. Scores are computed transposed
(scoresT[m, q]) so the softmax denominator reduces along partitions via a
ones-augmented value matrix (extra column of 1.0 in v), and the PV matmul
chains directly off the exp output.

All matmul operands are bf16 (PSUM accumulation stays fp32): halves HBM
traffic so the projections are never DMA-starved, and enables fast weight
load. exp runs on the Scalar engine from fp32 PSUM. All inputs are made
SBUF-resident with bulk DMAs issued up front.
"""

import numpy as np

import concourse.bass as bass
import concourse.mybir as mybir
import concourse.tile as tile
from concourse import bacc
from concourse.bass_utils import run_bass_kernel_spmd

B, N, DIM, H, DH = 2, 2048, 1024, 16, 64
NCORES = 8
HG = 4            # head-groups (cores per batch)
HPG = H // HG     # heads per group = 4
CS = DIM // HG    # channels per group = 256
P = 128
KT = DIM // P     # 8 contraction tiles for the projections
NT = N // P       # 16 token tiles
QW = 512          # q-chunk width (PSUM bank limit for fp32 accumulation)
QC = N // QW      # 4 q-chunks

FP32 = mybir.dt.float32
BF16 = mybir.dt.bfloat16
AF = mybir.ActivationFunctionType


def _build(scale: float, add_qk_bias: bool, reps: int = 1,
           loop_reps: int | None = None):
    nc = bacc.Bacc("TRN2", target_bir_lowering=False, debug=False,
                   num_devices=NCORES)

    qT = nc.dram_tensor("qT", [DIM, N], BF16, kind="ExternalInput").ap()
    kT = nc.dram_tensor("kT", [DIM, N], BF16, kind="ExternalInput").ap()
    vT = nc.dram_tensor("vT", [DIM, N], BF16, kind="ExternalInput").ap()
    wq = nc.dram_tensor("wq", [DIM, CS], BF16, kind="ExternalInput").ap()
    wk = nc.dram_tensor("wk", [DIM, CS], BF16, kind="ExternalInput").ap()
    wv = nc.dram_tensor("wv", [DIM, CS], BF16, kind="ExternalInput").ap()
    wp = nc.dram_tensor("wp", [CS, DIM], BF16, kind="ExternalInput").ap()
    bqk = nc.dram_tensor("bqk", [P, 2 * (CS // P)], FP32,
                         kind="ExternalInput").ap()
    out = nc.dram_tensor("out", [DIM, N], FP32, kind="ExternalOutput").ap()

    from contextlib import ExitStack
    with nc.allow_low_precision(reason="bf16 matmul rounding is intended"), \
         tile.TileContext(nc) as tc, ExitStack() as stack:
        wpool = stack.enter_context(tc.tile_pool(name="wpool", bufs=1))
        inres = stack.enter_context(tc.tile_pool(name="inres", bufs=1))
        persist = stack.enter_context(tc.tile_pool(name="persist", bufs=1))
        const = stack.enter_context(tc.tile_pool(name="const", bufs=1))

        # Weights resident in SBUF.
        wq_sb = wpool.tile([P, KT * CS], BF16, tag="wq")
        wk_sb = wpool.tile([P, KT * CS], BF16, tag="wk")
        wv_sb = wpool.tile([P, KT * CS], BF16, tag="wv")
        wp_sb = wpool.tile([P, (CS // P) * DIM], BF16, tag="wp")
        for k in range(KT):
            nc.sync.dma_start(out=wq_sb[:, k * CS:(k + 1) * CS],
                              in_=wq[k * P:(k + 1) * P, :])
            nc.sync.dma_start(out=wk_sb[:, k * CS:(k + 1) * CS],
                              in_=wk[k * P:(k + 1) * P, :])
            nc.sync.dma_start(out=wv_sb[:, k * CS:(k + 1) * CS],
                              in_=wv[k * P:(k + 1) * P, :])
        for k2 in range(CS // P):
            nc.sync.dma_start(out=wp_sb[:, k2 * DIM:(k2 + 1) * DIM],
                              in_=wp[k2 * P:(k2 + 1) * P, :])
        bqk_sb = const.tile([P, 2 * (CS // P)], FP32, tag="bqk")
        if add_qk_bias:
            nc.sync.dma_start(out=bqk_sb[:], in_=bqk[:])
        ones = const.tile([1, DH], BF16, tag="ones")
        nc.vector.memset(ones[:], 1.0)

        # Full activations resident in SBUF, channel-major, bf16.
        qres = [inres.tile([P, N], BF16, tag=f"qres{k}", name=f"qres{k}") for k in range(KT)]
        kres = [inres.tile([P, N], BF16, tag=f"kres{k}", name=f"kres{k}") for k in range(KT)]
        vres = [inres.tile([P, N], BF16, tag=f"vres{k}", name=f"vres{k}") for k in range(KT)]
        # Input DMAs issued from otherwise-idle engine queues so the
        # descriptor issues (~0.6us each) run in parallel with the weight
        # loads on the sync queue instead of serializing behind them.
        for k in range(KT):
            nc.scalar.dma_start(out=qres[k][:], in_=qT[k * P:(k + 1) * P, :])
        for k in range(KT):
            nc.gpsimd.dma_start(out=kres[k][:], in_=kT[k * P:(k + 1) * P, :])
        for k in range(KT):
            nc.gpsimd.dma_start(out=vres[k][:], in_=vT[k * P:(k + 1) * P, :])

        # Projected activations, channel-major, bf16.
        qsb = [persist.tile([P, N], BF16, tag=f"qsb{t}", name=f"qsb{t}") for t in range(2)]
        ksb = [persist.tile([P, N], BF16, tag=f"ksb{t}", name=f"ksb{t}") for t in range(2)]
        # v token-major with a ones column per head: [tok, 4*(64+1)]
        vsb = [persist.tile([P, HPG * (DH + 1)], BF16, tag=f"vsb{t}", name=f"vsb{t}")
               for t in range(NT)]

        from contextlib import nullcontext
        loop_cm = (tc.For_i(0, loop_reps, 1) if loop_reps
                   else nullcontext())
        with loop_cm:
          for rep in range(reps):
            # ---- Phase Q / K: channel-major projections -------------------
            def qk_proj(res, w_sb, dst, bias_col):
                with tc.tile_pool(name="pp", bufs=1, space="PSUM") as pp:
                    pA = pp.tile([P, N], FP32, tag="pA")
                    pB = pp.tile([P, N], FP32, tag="pB")
                    for k in range(KT):
                        for nn in range(QC):
                            nc.tensor.matmul(
                                pA[:, nn * QW:(nn + 1) * QW],
                                w_sb[:, k * CS:k * CS + P],
                                res[k][:, nn * QW:(nn + 1) * QW],
                                start=(k == 0), stop=(k == KT - 1))
                            nc.tensor.matmul(
                                pB[:, nn * QW:(nn + 1) * QW],
                                w_sb[:, k * CS + P:(k + 1) * CS],
                                res[k][:, nn * QW:(nn + 1) * QW],
                                start=(k == 0), stop=(k == KT - 1))
                    for t, pt in enumerate((pA, pB)):
                        if add_qk_bias:
                            nc.vector.tensor_scalar(
                                dst[t][:], pt[:],
                                bqk_sb[:, bias_col + t:bias_col + t + 1], None,
                                mybir.AluOpType.add)
                        else:
                            nc.vector.tensor_copy(dst[t][:], pt[:])

            qk_proj(kres, wk_sb, ksb, CS // P)

            # ---- Phase C: attention + output projection, per q-chunk ------
            # Software-pipelined: scores/exp run LAG m-tiles ahead of the PV
            # matmuls so the Scalar engine (the bottleneck: 128 exps of
            # [128,1024]) never starves at hp/qq boundaries. The softmax
            # division chain is DVE+GpSimd only (no PE instruction), and the
            # output projection of chunk qq is injected into chunk qq+1's
            # m-loop, with its PSUM coming from the sc ring.
            LAG = 3
            with tc.tile_pool(name="probs", bufs=LAG + 2) as probs, \
                 tc.tile_pool(name="xq", bufs=2) as xqp, \
                 tc.tile_pool(name="small", bufs=2) as small, \
                 tc.tile_pool(name="ost", bufs=2) as ostp, \
                 tc.tile_pool(name="psc", bufs=3, space="PSUM") as psc, \
                 tc.tile_pool(name="pxt", bufs=1, space="PSUM") as pxt:

                def oproj_tile(xqp_, qq_, mo):
                    qs_ = slice(qq_ * QW, (qq_ + 1) * QW)
                    po = psc.tile([P, 2 * QW], FP32, tag="sc",
                                  name=f"po_{qq_}_{mo}_{rep}")
                    for k2 in range(CS // P):
                        nc.tensor.matmul(
                            po[:, 0:QW],
                            wp_sb[:, k2 * DIM + mo * P:k2 * DIM + (mo + 1) * P],
                            xqp_[k2][:],
                            start=(k2 == 0), stop=(k2 == CS // P - 1))
                    ost = ostp.tile([P, QW], FP32, tag="ost")
                    nc.vector.tensor_copy(ost[:], po[:, 0:QW])
                    nc.sync.dma_start(out=out[mo * P:(mo + 1) * P, qs_],
                                      in_=ost[:])

                def qproj_chunk(c):
                    # Q projection for one 512-wide chunk, into a psc-ring
                    # tile: lets attention start right after K/V proj, with
                    # later chunks projected inside the previous chunk's
                    # m-loop where the PE has slack.
                    pq = psc.tile([P, 2 * QW], FP32, tag="sc",
                                  name=f"pq_{c}_{rep}")
                    for k in range(KT):
                        nc.tensor.matmul(
                            pq[:, 0:QW],
                            wq_sb[:, k * CS:k * CS + P],
                            qres[k][:, c * QW:(c + 1) * QW],
                            start=(k == 0), stop=(k == KT - 1))
                        nc.tensor.matmul(
                            pq[:, QW:2 * QW],
                            wq_sb[:, k * CS + P:(k + 1) * CS],
                            qres[k][:, c * QW:(c + 1) * QW],
                            start=(k == 0), stop=(k == KT - 1))
                    for t in range(2):
                        dst = qsb[t][:, c * QW:(c + 1) * QW]
                        pqs = pq[:, t * QW:(t + 1) * QW]
                        if add_qk_bias:
                            nc.vector.tensor_scalar(
                                dst, pqs, bqk_sb[:, t:t + 1], None,
                                mybir.AluOpType.add)
                        else:
                            nc.vector.tensor_copy(dst, pqs)

                def vproj_pair(p):
                    # V projection for token tiles 2p/2p+1: each psc-ring
                    # tile's two banks hold two accumulation groups
                    # (start=True clears a whole bank). Injected into
                    # qq=0/hp=0's m-loop: pair m lands well before PV
                    # consumes vsb[2m], so attention starts right after the
                    # K projection.
                    pv = psc.tile([P, 2 * QW], FP32, tag="sc",
                                  name=f"pvp_{p}_{rep}")
                    for u in range(2):
                        tt = 2 * p + u
                        for k in range(KT):
                            nc.tensor.matmul(
                                pv[:, u * QW:u * QW + CS],
                                vres[k][:, tt * P:(tt + 1) * P],
                                wv_sb[:, k * CS:(k + 1) * CS],
                                start=(k == 0), stop=(k == KT - 1))
                    for u in range(2):
                        tt = 2 * p + u
                        dst3 = vsb[tt][:].rearrange("p (h c) -> p h c", h=HPG)
                        nc.vector.tensor_copy(
                            dst3[:, :, 0:DH],
                            pv[:, u * QW:u * QW + CS].rearrange(
                                "p (h c) -> p h c", h=HPG))
                        nc.vector.memset(dst3[:, :, DH:DH + 1], 1.0)

                def emit_den(xtA, xtB, xq_, hp_):
                    # softmax division: den row -> reciprocal -> broadcast
                    # matmul across the head dim -> multiply. Emitted lazily
                    # (next hp's m==1) so the bc matmul's wait on the DVE
                    # chain never stalls the next scores in the PE FIFO.
                    for xt, off in ((xtA, 0), (xtB, DH)):
                        den = small.tile([1, QW], FP32, tag="den")
                        nc.vector.tensor_copy(den[:], xt[DH:DH + 1, :])
                        rde = small.tile([1, QW], FP32, tag="rde")
                        nc.vector.reciprocal_approx_fast(out=rde[:], in_=den[:])
                        rdr = small.tile([1, QW], BF16, tag="rdr")
                        nc.vector.tensor_copy(rdr[:], rde[:])
                        bc = psc.tile([P, 2 * QW], FP32, tag="sc")
                        nc.tensor.matmul(bc[0:DH, 0:QW], ones[:], rdr[:],
                                         start=True, stop=True)
                        bcs = small.tile([DH, QW], BF16, tag="bcs")
                        nc.vector.tensor_copy(bcs[:], bc[0:DH, 0:QW])
                        nc.vector.tensor_mul(xq_[hp_][off:off + DH, :],
                                             xt[0:DH, :], bcs[:])

                pend_out = None   # xq pair + qq whose out-proj is pending
                pend_den = None   # xt accumulators awaiting the division
                qproj_chunk(0)
                for qq in range(QC):
                    qs = slice(qq * QW, (qq + 1) * QW)
                    xq = [xqp.tile([P, QW], BF16, tag=f"x{t}", name=f"xq{t}_{qq}_{rep}") for t in range(2)]
                    for hp in range(HPG // 2):
                        # heads A = 2*hp (partitions 0:64 of tile hp),
                        # B = 2*hp+1 (partitions 64:128); their K=64 score
                        # matmuls occupy disjoint PE row-groups and run
                        # concurrently, sharing one [128, 1024] psum tile.
                        xtA = pxt.tile([P, QW], FP32, tag="xtA")
                        xtB = pxt.tile([P, QW], FP32, tag="xtB")
                        prq = []
                        for m in range(NT + LAG):
                            if m < NT:
                                sc = psc.tile([P, 2 * QW], FP32, tag="sc")
                                pr = probs.tile([P, 2 * QW], BF16, tag="pr")
                                prq.append(pr)
                                for j, off in ((0, 0), (1, DH)):
                                    nc.tensor.matmul(
                                        sc[:, j * QW:(j + 1) * QW],
                                        ksb[hp][off:off + DH, m * P:(m + 1) * P],
                                        qsb[hp][off:off + DH, qs],
                                        start=True, stop=True,
                                        tile_position=(off, 0))
                                nc.scalar.activation(pr[:], sc[:], AF.Exp,
                                                     scale=scale)
                            if m == 1 and pend_den is not None:
                                emit_den(*pend_den)
                                pend_den = None
                            if hp == 0 and pend_out is not None \
                                    and 4 <= m < 4 + KT:
                                oproj_tile(*pend_out, m - 4)
                                if m - 4 == KT - 1:
                                    pend_out = None
                            if qq == 0 and hp == 0 and m < NT // 2:
                                vproj_pair(m)
                            if hp == 1 and m == 13 and qq + 1 < QC:
                                qproj_chunk(qq + 1)
                            if m >= LAG:
                                mm = m - LAG
                                for j, xt, h in ((0, xtA, 2 * hp),
                                                 (1, xtB, 2 * hp + 1)):
                                    nc.tensor.matmul(
                                        xt[0:DH + 1, :],
                                        vsb[mm][:, h * (DH + 1):(h + 1) * (DH + 1)],
                                        prq[mm][:, j * QW:(j + 1) * QW],
                                        start=(mm == 0), stop=(mm == NT - 1))
                        pend_den = (xtA, xtB, xq, hp)
                    pend_out = (xq, qq)
                # tail: last hp's division, then last chunk's out-proj
                emit_den(*pend_den)
                pend_den = None
                for mo in range(KT):
                    oproj_tile(*pend_out, mo)
                pend_out = None

    nc.compile()
    return nc


_CACHE = {}


def _get_program(scale: float, add_qk_bias: bool, reps: int = 1,
                 loop_reps=None):
    key = (scale, add_qk_bias, reps, loop_reps)
    if key not in _CACHE:
        _CACHE[key] = _build(scale, add_qk_bias, reps, loop_reps)
    return _CACHE[key]


def make_in_maps(query, key, value, Wq, bq, Wk, bk, Wv, bv, Wp, bp, scale):
    import ml_dtypes
    bf16 = ml_dtypes.bfloat16
    query = np.asarray(query, np.float32)
    key = np.asarray(key, np.float32)
    value = np.asarray(value, np.float32)
    Wq, Wk, Wv, Wp = (np.asarray(a, np.float32) for a in (Wq, Wk, Wv, Wp))
    bq, bk = np.asarray(bq, np.float32), np.asarray(bk, np.float32)
    qkvT = [np.ascontiguousarray(a.transpose(0, 2, 1)).astype(bf16)
            for a in (query, key, value)]
    in_maps = []
    for c in range(NCORES):
        b, g = c // HG, c % HG
        cs = slice(g * CS, (g + 1) * CS)
        bqk_arr = np.stack([bq[cs].reshape(CS // P, P),
                            bk[cs].reshape(CS // P, P)]).reshape(-1, P).T
        in_maps.append({
            "qT": qkvT[0][b],
            "kT": qkvT[1][b],
            "vT": qkvT[2][b],
            "wq": np.ascontiguousarray(Wq[cs, :].T).astype(bf16),
            "wk": np.ascontiguousarray(Wk[cs, :].T).astype(bf16),
            "wv": np.ascontiguousarray(Wv[cs, :].T).astype(bf16),
            "wp": np.ascontiguousarray(Wp[:, cs].T).astype(bf16),
            "bqk": np.ascontiguousarray(bqk_arr),
        })
    return in_maps


def combine_outputs(results, bv, bp, Wp):
    bv = np.asarray(bv, np.float32)
    bp = np.asarray(bp, np.float32)
    Wp = np.asarray(Wp, np.float32)
    out = np.empty((B, N, DIM), np.float32)
    corr = bp + bv @ Wp.T
    for b in range(B):
        acc = np.asarray(results[b * HG]["out"], np.float32).copy()
        for g in range(1, HG):
            acc += np.asarray(results[b * HG + g]["out"], np.float32)
        out[b] = acc.T + corr
    return out


def kernel(query, key, value, Wq, bq, Wk, bk, Wv, bv, Wp, bp, scale):
    scale_v = float(np.asarray(scale).reshape(-1)[0])
    add_qk_bias = bool(np.any(np.asarray(bq)) or np.any(np.asarray(bk)))
    nc = _get_program(scale_v, add_qk_bias)
    in_maps = make_in_maps(query, key, value, Wq, bq, Wk, bk, Wv, bv,
                           Wp, bp, scale)
    res = run_bass_kernel_spmd(nc, in_maps, list(range(NCORES))).results
    return combine_outputs(res, bv, bp, Wp)
